# revision 48
# baseline (speedup 1.0000x reference)
"""Trainium2 Bass kernel for a dense transformer block (B=4,S=1024,D=1024,F=4096,H=16).

Sharding: 8 cores = (batch b in 0..3) x (seq half). Pure SPMD, no collectives:
the host rotates each core's tokens so its 512 query rows are always rows
0..511 of the rotated sequence; K/V cover the full (rotated) sequence.

Layout strategy per core:
  - LN1 token-major (bn_stats), output bf16, PE-transposed to feature-major y1T.
  - q^T/k^T feature-major via matmul(lhsT=W chunk, rhs=y1T);
    V token-major via matmul(lhsT=y1T chunk, rhs=W) with a fused ones column
    (V_aug [tok, 16, 65]) so the softmax row-sum rides the ctx matmul.
  - scores computed key-major: s^T[k,q] = matmul(lhsT=k^T_h, rhs=q^T_h), exp on
    ScalarE (per-(b,h) bias folded into the activation bias), multiplicative
    {0,1} mask on VectorE, ctx^T + rowsum = matmul(lhsT=V_aug, rhs=E).
  - normalization: recip(rowsum) then fp32r ones-matmul partition-broadcast.
  - Wo/FFN2 token-major outputs (lhsT=activation chunk, rhs=W chunk).
  - LN gains/biases and the q 1/sqrt(d) scale are folded into weights on host.
All matmuls run in bf16 with fp32 PSUM accumulation.
"""

import numpy as np
import ml_dtypes

import concourse.bass as bass
import concourse.mybir as mybir
import concourse.tile as tile
from concourse import bacc
from concourse.bass_utils import run_bass_kernel_spmd

F32 = mybir.dt.float32
F32R = mybir.dt.float32r
BF16 = mybir.dt.bfloat16
F8 = mybir.dt.float8e4
BF = ml_dtypes.bfloat16
F8NP = ml_dtypes.float8_e4m3
DR = mybir.MatmulPerfMode.DoubleRow

B, S, D, F, H = 4, 1024, 1024, 4096, 16
d = D // H          # 64
P = 128             # partitions
SQ = 512            # queries per core
EPS = 1e-5
NT = S // P         # 8 token tiles (full seq)
NQ = SQ // P        # 4 query tiles
NF = D // P         # 8 feature chunks
NF1 = F // P        # 32 ffn chunks

AX = mybir.AxisListType
ALU = mybir.AluOpType
ACTF = mybir.ActivationFunctionType


def _T(pool, shape, dtype, tag):
    return pool.tile(shape, dtype, name=tag, tag=tag)


def _pbcast(ap, p):
    """Partition-broadcast a [1, N] DRAM AP to [p, N]."""
    return bass.AP(tensor=ap.tensor, offset=ap.offset, ap=[[0, p]] + list(ap.ap[1:]))


def _build_program(FL, SC, reps=1):
    nc = bacc.Bacc("TRN2", target_bir_lowering=False, debug=False)

    t = {}
    t["x"] = nc.dram_tensor("x", [S, D], F32, kind="ExternalInput").ap()
    t["maskT"] = nc.dram_tensor("maskT", [NT // 2, P, 2, SQ], BF16,
                                kind="ExternalInput").ap()
    t["bias"] = nc.dram_tensor("bias", [1, H], F32, kind="ExternalInput").ap()
    t["wq"] = nc.dram_tensor("wq", [NF, P, NF, P], F8, kind="ExternalInput").ap()
    t["wk"] = nc.dram_tensor("wk", [NF, P, NF, P], F8, kind="ExternalInput").ap()
    t["wv"] = nc.dram_tensor("wv", [NF // 2, P, 2, D], F8, kind="ExternalInput").ap()
    t["wo"] = nc.dram_tensor("wo", [NF, P, 2, SQ], F8, kind="ExternalInput").ap()
    t["w1"] = nc.dram_tensor("w1", [NF1, P, D], BF16, kind="ExternalInput").ap()
    t["w2"] = nc.dram_tensor("w2", [2 * NF1, P, SQ], BF16, kind="ExternalInput").ap()
    t["bq"] = nc.dram_tensor("bq", [1, D], BF16, kind="ExternalInput").ap()
    t["bk"] = nc.dram_tensor("bk", [1, D], BF16, kind="ExternalInput").ap()
    t["bv"] = nc.dram_tensor("bv", [1, D], BF16, kind="ExternalInput").ap()
    t["bo"] = nc.dram_tensor("bo", [1, D], F32, kind="ExternalInput").ap()
    t["b1"] = nc.dram_tensor("b1", [1, F], BF16, kind="ExternalInput").ap()
    t["b2"] = nc.dram_tensor("b2", [1, D], BF16, kind="ExternalInput").ap()
    t["ident"] = nc.dram_tensor("ident", [P, P], BF16, kind="ExternalInput").ap()
    t["out"] = nc.dram_tensor("out", [SQ, D], F32, kind="ExternalOutput").ap()

    with tile.TileContext(nc) as tc:
        for rep in range(reps):
            _trace(nc, tc, t, FL, SC, pfx=f"r{rep}_" if reps > 1 else "")
    nc.compile()
    return nc


def _layernorm_tile(nc, pool, x_ap, out_ap, epst):
    """out = (x - mean(x)) * rsqrt(var(x) + eps) along the 1024-wide free dim."""
    st = _T(pool, [P, 2, 6], F32, "st")
    xr = x_ap.rearrange("p (a b) -> p a b", b=512)
    for sg in range(2):
        nc.vector.bn_stats(out=st[:, sg, :], in_=xr[:, sg, :])
    mv = _T(pool, [P, 2], F32, "mv")
    nc.vector.bn_aggr(out=mv[:], in_=st[:])
    rs = _T(pool, [P, 1], F32, "rs")
    nc.scalar.activation(out=rs[:], in_=mv[:, 1:2], func=ACTF.Sqrt,
                         bias=epst[:], scale=1.0)
    nc.vector.reciprocal(out=rs[:], in_=rs[:])
    nc.vector.tensor_scalar(out=out_ap, in0=x_ap, scalar1=mv[:, 0:1],
                            scalar2=rs[:], op0=ALU.subtract, op1=ALU.mult)


def _trace(nc, tc, t, FL, SC, pfx=""):
    x3 = t["x"].rearrange("(n p) c -> n p c", p=P)
    out3 = t["out"].rearrange("(n p) c -> n p c", p=P)

    with (
        tc.tile_pool(name=pfx + "const", bufs=1) as const_p,
        tc.tile_pool(name=pfx + "small", bufs=3) as small_p,
        tc.tile_pool(name=pfx + "xres", bufs=NQ) as xres_p,
        tc.tile_pool(name=pfx + "ctxT", bufs=NF) as ctxT_p,
        tc.tile_pool(name=pfx + "x2", bufs=NQ) as x2_p,
        tc.tile_pool(name=pfx + "wo", bufs=2 * NF) as wo_p,
        tc.tile_pool(name=pfx + "pt", bufs=2, space=bass.MemorySpace.PSUM) as ps_tp,
    ):
        # ---- constants ----
        ident = _T(const_p, [P, P], BF16, "ident")
        nc.sync.dma_start(out=ident[:], in_=t["ident"])
        if any(FL[k] for k in ("bq", "bk", "bv", "b1", "b2")):
            ones = _T(const_p, [1, SQ], BF16, "ones")
            nc.vector.memset(ones[:], 1.0)
        epst = _T(const_p, [P, 1], F32, "epst")
        nc.vector.memset(epst[:], EPS)
        if FL["bo"]:
            bo_b = _T(const_p, [P, D], F32, "bo_b")
            nc.sync.dma_start(out=bo_b[:], in_=_pbcast(t["bo"], P))
        brow = {}
        for nm in ("bq", "bk", "bv", "b1", "b2"):
            if not FL[nm]:
                continue
            sz = F if nm == "b1" else D
            brow[nm] = _T(const_p, [1, sz], BF16, f"brow_{nm}")
            nc.sync.dma_start(out=brow[nm][:], in_=t[nm])

        xres = [_T(xres_p, [P, D], F32, "xres") for _ in range(NQ)]
        # ctxT fp8 pairs [P, 2, SQ] (dim1 = fi-chunk pair member) for the
        # DoubleRow Wo matmul; carries the SCTX scale (ones-column trick)
        ctxT = [_T(ctxT_p, [P, 2, SQ], F8, "ctxT") for _ in range(NF // 2)]
        x2 = [_T(x2_p, [P, D], F32, "x2") for _ in range(NQ)]

        # ================= phase 1: LN1, y1T, QKV, attention =================
        with (
            tc.tile_pool(name=pfx + "y1T", bufs=NF // 2) as y1T_p,
            tc.tile_pool(name=pfx + "kT", bufs=NF) as kT_p,
            tc.tile_pool(name=pfx + "qT", bufs=NF) as qT_p,
            tc.tile_pool(name=pfx + "mk", bufs=NT // 2) as mk_p,
        ):
            # y1T pairs: [P, 2, S] fp8 — dim1 is the fi-chunk pair member for
            # DoubleRow matmuls (chunk 2g+j lives at y1T[g][:, j, :]).
            y1T = [_T(y1T_p, [P, 2, S], F8, "y1T") for _ in range(NF // 2)]
            qT = [_T(qT_p, [P, SQ], BF16, "qT") for _ in range(NF)]
            kT = [_T(kT_p, [P, S], BF16, "kT") for _ in range(NF)]
            V = [_T(kT_p, [P, H, 2 * d], BF16, "V") for _ in range(NT)]
            mT = [_T(mk_p, [P, 2, SQ], BF16, "mk") for _ in range(NT // 2)]

            # ---- phase 1a: LN1 + projections (q, k, V all upfront) ----
            with (
                tc.tile_pool(name=pfx + "xs", bufs=NQ) as xs_p,
                tc.tile_pool(name=pfx + "y1", bufs=2) as y1_p,
                tc.tile_pool(name=pfx + "wqk", bufs=3) as wqk_p,
                tc.tile_pool(name=pfx + "wv", bufs=NF // 2) as wv_p,
                tc.tile_pool(name=pfx + "pm", bufs=3,
                             space=bass.MemorySpace.PSUM) as ps_mm,
            ):
                # all x tiles DMA'd upfront (sync queue, nothing ahead of
                # them) so the LN chain never waits on input data
                xall = list(xres) + [_T(xs_p, [P, D], F32, "xs")
                                     for _ in range(NT - NQ)]
                for tt in range(NT):
                    nc.sync.dma_start(out=xall[tt][:], in_=x3[tt])

                # prefetch Wo for phase 2 (Pool queue is idle during
                # attention; the DMA lands long before the first Wo matmul)
                wo_t = [_T(wo_p, [P, 2, SQ], F8, "woW") for _ in range(NF)]
                for i in range(NF):
                    nc.gpsimd.dma_start(out=wo_t[i][:], in_=t["wo"][i])

                def ln1_tile(tt):
                    xt = xall[tt]
                    yt = _T(y1_p, [P, D], BF16, "y1")
                    _layernorm_tile(nc, small_p, xt[:], yt[:], epst)
                    for fc in range(NF):
                        pt = _T(ps_tp, [P, P], BF16, "pt")
                        nc.tensor.transpose(pt[:], yt[:, fc * P:(fc + 1) * P],
                                            ident[:])
                        nc.scalar.copy(
                            out=y1T[fc // 2][:, fc % 2, tt * P:(tt + 1) * P],
                            in_=pt[:])

                # LN the own-query tiles first so q projections (which only
                # read y1T[:, :, 0:SQ]) can start while LN of tiles 4-7 runs
                for tt in range(NQ):
                    ln1_tile(tt)

                # q^T: [f_out 128, q 512] per chunk (only own 512 queries)
                for fo in range(NF):
                    wt = _T(wqk_p, [P, NF, P], F8, "wqk")
                    nc.gpsimd.dma_start(out=wt[:], in_=t["wq"][fo])
                    ps = _T(ps_mm, [P, SQ], F32, "pm")
                    ps = ps[:]
                    for fj in range(NF // 2):
                        nc.tensor.matmul(ps, wt[:, 2 * fj:2 * fj + 2, :],
                                         y1T[fj][:, :, 0:SQ],
                                         start=(fj == 0),
                                         stop=(not FL["bq"] and fj == NF // 2 - 1),
                                         perf_mode=DR)
                    if FL["bq"]:
                        nc.tensor.matmul(ps, brow["bq"][:, fo * P:(fo + 1) * P],
                                         ones[:], start=False, stop=True)
                    nc.vector.tensor_copy(out=qT[fo][:], in_=ps)
                    if fo < NQ:
                        ln1_tile(NQ + fo)

                # k^T: [f_out 128, tok 1024] per chunk (full sequence)
                for fo in range(NF):
                    wt = _T(wqk_p, [P, NF, P], F8, "wqk")
                    nc.gpsimd.dma_start(out=wt[:], in_=t["wk"][fo])
                    for th in range(2):
                        tsl = slice(th * SQ, (th + 1) * SQ)
                        ps = _T(ps_mm, [P, SQ], F32, "pm")
                        for fj in range(NF // 2):
                            nc.tensor.matmul(ps[:], wt[:, 2 * fj:2 * fj + 2, :],
                                             y1T[fj][:, :, tsl],
                                             start=(fj == 0),
                                             stop=(not FL["bk"] and
                                                   fj == NF // 2 - 1),
                                             perf_mode=DR)
                        if FL["bk"]:
                            nc.tensor.matmul(ps[:],
                                             brow["bk"][:, fo * P:(fo + 1) * P],
                                             ones[:], start=False, stop=True)
                        nc.vector.tensor_copy(out=kT[fo][:, tsl], in_=ps[:])

                # mask DMAs issued here (sync queue) — not needed until
                # attention, must not delay the x tiles above
                for kp in range(NT // 2):
                    nc.sync.dma_start(out=mT[kp][:], in_=t["maskT"][kp])

                # V token-major, heads interleaved d-cols then d ones-cols.
                # V carries the wv fp8 scale WSv; the ones columns are memset
                # to WSv so the softmax normalization cancels it exactly.
                wv_t = [_T(wv_p, [P, 2, D], F8, "wv") for _ in range(NF // 2)]
                for g in range(NF // 2):
                    nc.gpsimd.dma_start(out=wv_t[g][:], in_=t["wv"][g])
                for kt in range(NT):
                    nc.gpsimd.memset(V[kt][:, :, d:], SC["vones"])
                    for fh in range(2):
                        fsl = slice(fh * SQ, (fh + 1) * SQ)
                        ps = _T(ps_mm, [P, SQ], F32, "pm")
                        for g in range(NF // 2):
                            nc.tensor.matmul(ps[:],
                                             y1T[g][:, :, kt * P:(kt + 1) * P],
                                             wv_t[g][:, :, fsl],
                                             start=(g == 0),
                                             stop=(not FL["bv"] and
                                                   g == NF // 2 - 1),
                                             perf_mode=DR)
                        if FL["bv"]:
                            nc.tensor.matmul(ps[:], ones[:, 0:P],
                                             brow["bv"][:, fsl],
                                             start=False, stop=True)
                        # ACT evacuates V psum: this window is LN-free so DVE
                        # is the busy engine and ScalarE idles
                        nc.scalar.copy(
                            out=V[kt][:, fh * (H // 2):(fh + 1) * (H // 2), 0:d],
                            in_=ps[:].rearrange("p (a b) -> p a b", b=d))

            # ---- phase 1b: attention (scores/exp/mask/ctx) ----
            with (
                tc.tile_pool(name=pfx + "es", bufs=12) as e_p,
                tc.tile_pool(name=pfx + "sp", bufs=2,
                             space=bass.MemorySpace.PSUM) as ps_sp,
                tc.tile_pool(name=pfx + "pc", bufs=2,
                             space=bass.MemorySpace.PSUM) as ps_ctx,
            ):
                def score_kp(h, kp):
                    """scores^T -> exp -> mask for one (head, key-tile PAIR).
                    Two score matmuls land in one 2-bank psum tile; a single
                    exp and a single mask-multiply cover both key tiles
                    (activation per-instruction overhead is ~350ns, so
                    fewer+larger is cheaper). Even/odd heads sit at PE row
                    groups 0-63/64-127 and can run concurrently when adjacent
                    in the PE stream."""
                    fc, po = h // 2, (h % 2) * d
                    ps_s = _T(ps_sp, [P, 2, SQ], F32, "sp")
                    for j in range(2):
                        kt = 2 * kp + j
                        nc.tensor.matmul(ps_s[:, j, :],
                                         kT[fc][po:po + d, kt * P:(kt + 1) * P],
                                         qT[fc][po:po + d, :],
                                         start=True, stop=True)
                    e = _T(e_p, [P, 2, SQ], BF16, "e")
                    nc.scalar.activation(out=e[:], in_=ps_s[:], func=ACTF.Exp,
                                         scale=SC["exp"])
                    nc.vector.tensor_mul(e[:], e[:], mT[kp][:])
                    return e

                def ctx_mm(pcs, h, kp, e):
                    for j in range(2):
                        kt = 2 * kp + j
                        nc.tensor.matmul(pcs[:], V[kt][:, h, :], e[:, j, :],
                                         start=(kt == 0), stop=(kt == NT - 1),
                                         skip_group_check=True)

                def ctx_norm(h, pcs):
                    fco, po = h // 2, (h % 2) * d
                    rb = _T(small_p, [d, SQ], F32, "rb")
                    nc.vector.reciprocal(rb[:], pcs[d:2 * d, :])
                    nc.vector.tensor_mul(
                        ctxT[fco // 2][po:po + d, fco % 2, :], pcs[0:d, :],
                        rb[:])

                LAG = 2
                for fc in range(NF):
                    pair = (2 * fc, 2 * fc + 1)
                    es = {h: [] for h in pair}
                    pcs = {h: _T(ps_ctx, [P, SQ], F32, "pc") for h in pair}
                    for kp in range(NT // 2):
                        for h in pair:
                            es[h].append(score_kp(h, kp))
                        if kp >= LAG:
                            for h in pair:
                                ctx_mm(pcs[h], h, kp - LAG, es[h][kp - LAG])
                    for kp in range(NT // 2 - LAG, NT // 2):
                        for h in pair:
                            ctx_mm(pcs[h], h, kp, es[h][kp])
                    for h in pair:
                        ctx_norm(h, pcs[h])

        # ================= phase 2: Wo, LN2, FFN =================
        with (
            tc.tile_pool(name=pfx + "wh", bufs=10) as wh_p,
            tc.tile_pool(name=pfx + "w1s", bufs=2) as w1_p,
            tc.tile_pool(name=pfx + "hT", bufs=NF1) as hT_p,
            tc.tile_pool(name=pfx + "y2", bufs=2) as y2_p,
            tc.tile_pool(name=pfx + "y2T", bufs=NF) as y2T_p,
            tc.tile_pool(name=pfx + "xo", bufs=NQ) as xo_p,
            tc.tile_pool(name=pfx + "pm2", bufs=2, space=bass.MemorySpace.PSUM) as ps_mm,
            tc.tile_pool(name=pfx + "p4", bufs=4, space=bass.MemorySpace.PSUM) as ps_4,
        ):
            # Wo (fp8 DoubleRow; weights prefetched in phase 1a). The psum
            # carries SCTX*WSo, undone by the scaled evacuation on DVE.
            y2T = [_T(y2T_p, [P, SQ], BF16, "y2T") for _ in range(NF)]
            for qt in range(NQ):
                for dh in range(2):
                    dsl = slice(dh * SQ, (dh + 1) * SQ)
                    ps = _T(ps_mm, [P, SQ], F32, "pm")
                    for g in range(NF // 2):
                        nc.tensor.matmul(ps[:],
                                         ctxT[g][:, :, qt * P:(qt + 1) * P],
                                         wo_t[dh * (NF // 2) + g][:],
                                         start=(g == 0), stop=(g == NF // 2 - 1),
                                         perf_mode=DR)
                    # scale on ACT (idle here), residual add on DVE
                    nc.scalar.activation(out=x2[qt][:, dsl], in_=ps[:],
                                         func=ACTF.Copy, scale=SC["wo_evac"])
                    nc.vector.tensor_add(x2[qt][:, dsl], x2[qt][:, dsl],
                                         xres[qt][:, dsl])
                    if FL["bo"]:
                        nc.vector.tensor_add(x2[qt][:, dsl], x2[qt][:, dsl],
                                             bo_b[:, dsl])
                yt = _T(y2_p, [P, D], BF16, "y2")
                _layernorm_tile(nc, small_p, x2[qt][:], yt[:], epst)
                for fc in range(NF):
                    pt = _T(ps_tp, [P, P], BF16, "pt")
                    nc.tensor.transpose(pt[:], yt[:, fc * P:(fc + 1) * P], ident[:])
                    nc.scalar.copy(out=y2T[fc][:, qt * P:(qt + 1) * P], in_=pt[:])

            # FFN1: h^T[f1 128, q 512] = gelu(W1' y2 + b1')
            hT = [_T(hT_p, [P, SQ], BF16, "hT") for _ in range(NF1)]
            for f1 in range(NF1):
                wt = _T(w1_p, [P, D], BF16, "w1s")
                nc.gpsimd.dma_start(out=wt[:], in_=t["w1"][f1])
                ps = _T(ps_mm, [P, SQ], F32, "pm")
                for fi in range(NF):
                    nc.tensor.matmul(ps[:], wt[:, fi * P:(fi + 1) * P], y2T[fi][:],
                                     start=(fi == 0),
                                     stop=(not FL["b1"] and fi == NF - 1))
                if FL["b1"]:
                    nc.tensor.matmul(ps[:], brow["b1"][:, f1 * P:(f1 + 1) * P],
                                     ones[:], start=False, stop=True)
                nc.scalar.activation(out=hT[f1][:], in_=ps[:], func=ACTF.Gelu)

            # FFN2 + residual: out = x2 + h @ W2 + b2
            xout = [_T(xo_p, [P, D], F32, "xo") for _ in range(NQ)]
            for dh in range(2):
                dsl = slice(dh * SQ, (dh + 1) * SQ)
                ps4 = [_T(ps_4, [P, SQ], F32, "p4") for _ in range(NQ)]
                for f1 in range(NF1):
                    wt = _T(wh_p, [P, SQ], BF16, "wh")
                    nc.gpsimd.dma_start(out=wt[:], in_=t["w2"][dh * NF1 + f1])
                    for qt in range(NQ):
                        nc.tensor.matmul(ps4[qt][:],
                                         hT[f1][:, qt * P:(qt + 1) * P], wt[:],
                                         start=(f1 == 0),
                                         stop=(not FL["b2"] and f1 == NF1 - 1))
                for qt in range(NQ):
                    if FL["b2"]:
                        nc.tensor.matmul(ps4[qt][:], ones[:, 0:P],
                                         brow["b2"][:, dsl],
                                         start=False, stop=True)
                    nc.vector.tensor_add(xout[qt][:, dsl], ps4[qt][:],
                                         x2[qt][:, dsl])
                    nc.sync.dma_start(out=out3[qt][:, dsl],
                                      in_=xout[qt][:, dsl])


_NC = {}
_ALL_FLAGS = ("bq", "bk", "bv", "bo", "b1", "b2")


def _get_nc(flags=None, scales=None, reps=1):
    if flags is None:
        flags = {k: True for k in _ALL_FLAGS}
    if scales is None:
        scales = {"exp": 1.0, "vones": 1.0}
    key = (tuple(sorted(flags.items())), tuple(sorted(scales.items())), reps)
    if key not in _NC:
        _NC[key] = _build_program(dict(flags), dict(scales), reps=reps)
    return _NC[key]


def _pow2_scale(w, target=224.0):
    """Power-of-2 scale putting absmax(w) near (but under) target (fp8e4m3
    max finite = 240)."""
    a = float(np.max(np.abs(w)))
    if a == 0.0:
        return 1.0
    return float(2.0 ** np.floor(np.log2(target / a)))


def _prep_inputs(inputs):
    """Host-side folding + per-core shard maps."""
    x = np.asarray(inputs["x"], np.float32)
    attn_bias = np.asarray(inputs["attn_bias"], np.float32)
    mask = np.asarray(inputs["mask"], np.float32)
    g1 = np.asarray(inputs["ln1_g"], np.float32)
    b1n = np.asarray(inputs["ln1_b"], np.float32)
    g2 = np.asarray(inputs["ln2_g"], np.float32)
    b2n = np.asarray(inputs["ln2_b"], np.float32)
    Wq = np.asarray(inputs["Wq"], np.float32); bq = np.asarray(inputs["bq"], np.float32)
    Wk = np.asarray(inputs["Wk"], np.float32); bk = np.asarray(inputs["bk"], np.float32)
    Wv = np.asarray(inputs["Wv"], np.float32); bv = np.asarray(inputs["bv"], np.float32)
    Wo = np.asarray(inputs["Wo"], np.float32); bo = np.asarray(inputs["bo"], np.float32)
    W1 = np.asarray(inputs["W1"], np.float32); b1 = np.asarray(inputs["b1"], np.float32)
    W2 = np.asarray(inputs["W2"], np.float32); b2 = np.asarray(inputs["b2"], np.float32)

    scale = d ** -0.5
    # fold LN gains/biases (and q scale) into the projection weights
    Wq_e = (g1[:, None] * Wq) * scale
    bq_e = (b1n @ Wq + bq) * scale
    Wk_e = g1[:, None] * Wk
    bk_e = b1n @ Wk + bk
    Wv_e = g1[:, None] * Wv
    bv_e = b1n @ Wv + bv
    W1_e = g2[:, None] * W1
    b1_e = b2n @ W1 + b1

    # fp8 power-of-2 weight scales; undone via exp/gelu pre-scales and the
    # V ones-column trick (softmax normalization cancels WSv exactly).
    WSq = _pow2_scale(Wq_e)
    WSk = _pow2_scale(Wk_e)
    WSv = _pow2_scale(Wv_e)
    WSo = _pow2_scale(Wo)
    SCTX = 32.0   # fp8 scale carried by ctxT (ctx absmax ~1.5; 32x < 240)
    scales = {
        "exp": 1.0 / (WSq * WSk),
        # ones columns produce rowsum*(WSv/SCTX) so ctxT = SCTX * ctx exactly
        "vones": float(WSv / SCTX),
        "wo_evac": float(1.0 / (SCTX * WSo)),
    }

    def tile_fo4(W, ws, n_out):
        # [Din, Dout] -> [n_out(fo), P(fi within chunk), NF(fi), P(fo within)]
        return np.ascontiguousarray(
            (W * ws).reshape(NF, P, n_out, P).transpose(2, 1, 0, 3)
        ).astype(F8NP)

    wq_h = tile_fo4(Wq_e, WSq, NF)
    wk_h = tile_fo4(Wk_e, WSk, NF)
    # Wv -> [NF//2 (fi pair), P(fi within chunk), 2(pair member), D(fo)]
    wv_h = np.ascontiguousarray(
        (Wv_e * WSv).reshape(NF // 2, 2, P, D).transpose(0, 2, 1, 3)
    ).astype(F8NP)
    # Wo -> [dh*4+g, P(fi within chunk), 2(pair member), SQ] fp8
    wo_h = np.ascontiguousarray(
        (Wo * WSo).reshape(NF // 2, 2, P, 2, SQ).transpose(3, 0, 2, 1, 4)
        .reshape(NF, P, 2, SQ)
    ).astype(F8NP)
    # W1 -> [NF1(f1), P(f_in within chunk), NF(fi)*P]
    w1_h = np.ascontiguousarray(
        W1_e.reshape(NF, P, NF1, P).transpose(2, 1, 0, 3).reshape(NF1, P, D)
    ).astype(BF)
    # W2 -> [2*NF1 (dh,f1), P, SQ]
    w2_h = np.ascontiguousarray(
        W2.reshape(NF1, P, 2, SQ).transpose(2, 0, 1, 3).reshape(2 * NF1, P, SQ)
    ).astype(BF)

    flags = {
        "bq": bool(np.any(bq_e)), "bk": bool(np.any(bk_e)),
        "bv": bool(np.any(bv_e)), "bo": bool(np.any(bo)),
        "b1": bool(np.any(b1_e)), "b2": bool(np.any(b2)),
    }
    shared = {
        "ident": np.eye(P, dtype=BF),
        "wq": wq_h, "wk": wk_h, "wv": wv_h, "wo": wo_h, "w1": w1_h, "w2": w2_h,
        "bq": (bq_e * WSq).reshape(1, D).astype(BF),
        "bk": (bk_e * WSk).reshape(1, D).astype(BF),
        "bv": (bv_e * WSv).reshape(1, D).astype(BF),
        "bo": bo.reshape(1, D).astype(np.float32),
        "b1": b1_e.reshape(1, F).astype(BF),
        "b2": b2.reshape(1, D).astype(BF),
    }

    in_maps = []
    for c in range(8):
        b, hf = c // 2, c % 2
        x_rot = np.ascontiguousarray(np.roll(x[b], -SQ * hf, axis=0))
        mq = mask[b, 0, SQ * hf:SQ * (hf + 1), :]          # [512 q, 1024 k]
        mT = np.ascontiguousarray(
            np.roll(mq.T, -SQ * hf, axis=0).reshape(NT // 2, 2, P, SQ)
            .transpose(0, 2, 1, 3)).astype(BF)
        bias_c = (attn_bias[b - 1] if b > 0 else np.zeros(H, np.float32))
        m = dict(shared)
        m["x"] = x_rot
        m["maskT"] = mT
        m["bias"] = bias_c.reshape(1, H).astype(np.float32)
        in_maps.append(m)
    return in_maps, flags, scales


def run(inputs, trace=False, **kw):
    in_maps, flags, scales = _prep_inputs(inputs)
    nc = _get_nc(flags, scales)
    res = run_bass_kernel_spmd(nc, in_maps, core_ids=list(range(8)),
                               trace=trace, **kw)
    out = np.empty((B, S, D), np.float32)
    for c in range(8):
        b, hf = c // 2, c % 2
        out[b, SQ * hf:SQ * (hf + 1), :] = res.results[c]["out"]
    return out, res


def kernel(**inputs) -> np.ndarray:
    out, _ = run(inputs, trace=False)
    return out

